# revision 3
# baseline (speedup 1.0000x reference)
"""Trainium2 Bass kernel for nn_CONV_A_64115271795341 — v4.

Same math as v3 (3 taps per matmul via [x; x>>row] contraction pairing plus
a beta column-group computing kernel-row-2 partials 2 output rows behind),
restructured for pipeline overlap:

  - psum groups of SPG=2 spans (2 banks) x 4 pool buffers = all 8 banks,
    giving 3 groups of slack between matmuls and the drain chain.
  - beta partials staged contiguously into sbB[64, H*W + 2W] (one ACT copy
    per group, partition-crossing 64:128 -> 0:64); the DVE add then reads
    a +2-row shifted window of sbB, no separate boundary ops.
  - cleanup matmuls (N=256) produce the last 2 rows' beta partials into a
    recycled psum buffer, staged into sbB's tail.
  - out[q] = psTop[q] + sbB[q+2 rows]; outt fp16, 2 big output DMAs.
"""

import numpy as np

C, H, W, D, B = 64, 128, 128, 64, 8
KS = 3
WP = W + 2            # 130
HP = H + 2
NP = WP * HP          # 16900
TILE_ROWS = 4
TN = TILE_ROWS * W    # 512
NSPANS = H // TILE_ROWS   # 32
SPG = 2                   # spans per psum group
NG = NSPANS // SPG        # 16 groups
G = SPG * TN              # 1024 cols per group
SHIFT = 2 * W             # 256: beta partials lag 2 output rows

_CACHE = {}


def _build(repeat=1, in_chunks=4, out_chunks=1, psum_bufs=4, xp_bufs=2,
           out_bufs=2, stg_dtype="float32", dup="host", dup_chunks=8,
           skip_in=False, skip_out=False, skip_drain=False, skip_mm=False):
    import concourse.bass as bass  # noqa: F401
    import concourse.mybir as mybir
    import concourse.tile as tile
    from concourse import bacc

    dt = mybir.dt
    sdt = getattr(dt, stg_dtype)
    nc = bacc.Bacc("TRN2", target_bir_lowering=False, debug=False, num_devices=8)

    x_d = nc.dram_tensor("x", [128, NP], dt.float16, kind="ExternalInput")
    w_d = nc.dram_tensor("w", [128, KS * 128], dt.float16, kind="ExternalInput")
    out_d = nc.dram_tensor("out", [D, H * W], dt.float16, kind="ExternalOutput")

    with tile.TileContext(nc) as tc:
        with tc.tile_pool(name="io", bufs=xp_bufs) as io_pool, \
             tc.tile_pool(name="wp", bufs=2) as w_pool, \
             tc.tile_pool(name="outp", bufs=out_bufs) as out_pool, \
             tc.tile_pool(name="stg", bufs=1) as stg_pool, \
             tc.tile_pool(name="ps", bufs=psum_bufs, space="PSUM") as ps_pool:

            for _rep in range(repeat):
                w_sb = w_pool.tile([128, KS * 128], dt.float16,
                                   name="w_sb", tag="w_sb")
                nc.sync.dma_start(w_sb[:, :], w_d.ap()[:, :])

                xp = io_pool.tile([128, NP], dt.float16, name="xp", tag="xp")
                bnd = [NP * g // in_chunks for g in range(in_chunks + 1)]
                if not skip_in:
                    for g in range(in_chunks):
                        a, b = bnd[g], bnd[g + 1]
                        nc.sync.dma_start(xp[:, a:b], x_d.ap()[:, a:b])
                else:
                    nc.sync.dma_start(xp[:, 0:NP], x_d.ap()[:, 0:NP]) if False else                     nc.sync.dma_start(xp[:, 0:64], x_d.ap()[:, 0:64])

                xv = xp.rearrange("p (r c) -> p r c", c=WP)
                outt = sbB = None
                if not skip_drain:
                    outt = out_pool.tile([D, H * W], dt.float16,
                                         name="outt", tag="outt")
                    sbB = stg_pool.tile([64, H * W + SHIFT], sdt,
                                        name="sbB", tag="sbB")

                ps_list = []
                for g in range(NG):
                    if skip_mm:
                        break
                    psP = ps_pool.tile([128, G], mybir.dt.float32,
                                       name="psP", tag="psP")
                    ps_list.append(psP)
                    for s in range(SPG if not skip_mm else 0):
                        h0 = TILE_ROWS * (SPG * g + s)
                        for j in range(KS):
                            nc.tensor.matmul(
                                psP[:, TN * s:TN * (s + 1)],
                                lhsT=w_sb[:, 128 * j:128 * (j + 1)],
                                rhs=xv[:, h0:h0 + TILE_ROWS, j:j + W],
                                start=(j == 0), stop=(j == KS - 1),
                            )
                    # stage this group's beta partials contiguously
                    if skip_drain:
                        continue
                    nc.scalar.copy(sbB[:, G * g:G * (g + 1)], psP[64:128, :])
                    if g > 0:
                        nc.vector.tensor_add(
                            outt[:, G * (g - 1):G * g],
                            ps_list[g - 1][0:64, :],
                            sbB[:, G * (g - 1) + SHIFT:G * g + SHIFT])
                # cleanup: beta partials for the last 2 output rows
                psQ = None
                if not (skip_drain or skip_mm):
                    psQ = ps_pool.tile([128, G], mybir.dt.float32,
                                       name="psP", tag="psP")
                for j in range(KS if not (skip_drain or skip_mm) else 0):
                    nc.tensor.matmul(
                        psQ[0:64, 0:SHIFT],
                        lhsT=w_sb[0:64, 128 * j + 64:128 * j + 128],
                        rhs=xv[0:64, H:H + 2, j:j + W],
                        start=(j == 0), stop=(j == KS - 1),
                    )
                if not skip_drain:
                    nc.scalar.copy(sbB[:, H * W:H * W + SHIFT],
                                   psQ[0:64, 0:SHIFT])
                    nc.vector.tensor_add(
                        outt[:, G * (NG - 1):G * NG],
                        ps_list[NG - 1][0:64, :],
                        sbB[:, G * (NG - 1) + SHIFT:G * NG + SHIFT])

                obnd = [H * W * g // out_chunks for g in range(out_chunks + 1)]
                osrc = xp[0:64, 0:H * W] if skip_drain else outt
                if not skip_out:
                    for g in range(out_chunks):
                        a, b = obnd[g], obnd[g + 1]
                        nc.scalar.dma_start(out_d.ap()[:, a:b], osrc[:, a:b])

    nc.compile()
    return nc


def _prep_inputs(x, weight, w_lin):
    w = np.asarray(weight).astype(np.float64)
    weff = w + (np.asarray(w_lin).astype(np.float64).T[:, None, :]
                - w.sum(axis=1, keepdims=True)) / 9.0
    weff = weff.astype(np.float32)                      # [C, 9, D]
    w_sb = np.zeros((128, KS * 128), np.float16)
    for j in range(KS):
        w_sb[0:C, 128 * j:128 * j + 64] = weff[:, 0 * KS + j, :]
        w_sb[C:128, 128 * j:128 * j + 64] = weff[:, 1 * KS + j, :]
        w_sb[0:C, 128 * j + 64:128 * j + 128] = weff[:, 2 * KS + j, :]

    xpad = np.pad(np.asarray(x), ((0, 0), (0, 0), (1, 1), (1, 1)), mode="edge")
    xpad = xpad.reshape(B, C, NP).astype(np.float16)
    xh = np.zeros((B, 128, NP), np.float16)
    xh[:, 0:C, :] = xpad
    xh[:, C:128, 0:NP - WP] = xpad[:, :, WP:]
    return xh, w_sb


def kernel(x, weight, w_lin):
    from concourse.bass_utils import run_bass_kernel_spmd

    if "nc" not in _CACHE:
        _CACHE["nc"] = _build()
    nc = _CACHE["nc"]

    xh, w_sb = _prep_inputs(x, weight, w_lin)
    in_maps = [{"x": xh[b], "w": w_sb} for b in range(B)]
    res = run_bass_kernel_spmd(nc, in_maps, core_ids=list(range(B)))
    out = np.stack([res.results[b]["out"].reshape(D, H, W) for b in range(B)])
    return out.astype(np.float32)


# revision 4
# speedup vs baseline: 1.0688x; 1.0688x over previous
"""Trainium2 Bass kernel for nn_CONV_A_64115271795341 — v4.

Same math as v3 (3 taps per matmul via [x; x>>row] contraction pairing plus
a beta column-group computing kernel-row-2 partials 2 output rows behind),
restructured for pipeline overlap:

  - psum groups of SPG=2 spans (2 banks) x 4 pool buffers = all 8 banks,
    giving 3 groups of slack between matmuls and the drain chain.
  - beta partials staged contiguously into sbB[64, H*W + 2W] (one ACT copy
    per group, partition-crossing 64:128 -> 0:64); the DVE add then reads
    a +2-row shifted window of sbB, no separate boundary ops.
  - cleanup matmuls (N=256) produce the last 2 rows' beta partials into a
    recycled psum buffer, staged into sbB's tail.
  - out[q] = psTop[q] + sbB[q+2 rows]; outt fp16, 2 big output DMAs.
"""

import numpy as np

C, H, W, D, B = 64, 128, 128, 64, 8
KS = 3
WP = W + 2            # 130
HP = H + 2
NP = WP * HP          # 16900
TILE_ROWS = 4
TN = TILE_ROWS * W    # 512
NSPANS = H // TILE_ROWS   # 32
SPG = 2                   # spans per psum group
NG = NSPANS // SPG        # 16 groups
G = SPG * TN              # 1024 cols per group
SHIFT = 2 * W             # 256: beta partials lag 2 output rows

_CACHE = {}


def _build(repeat=1, in_chunks=4, out_chunks=1, psum_bufs=4, xp_bufs=2,
           out_bufs=2, stg_dtype="float32", dup="host", dup_chunks=8,
           spg=SPG, skip_in=False, skip_out=False, skip_drain=False,
           skip_mm=False):
    NGl = NSPANS // spg
    Gl = spg * TN
    import concourse.bass as bass  # noqa: F401
    import concourse.mybir as mybir
    import concourse.tile as tile
    from concourse import bacc

    dt = mybir.dt
    sdt = getattr(dt, stg_dtype)
    nc = bacc.Bacc("TRN2", target_bir_lowering=False, debug=False, num_devices=8)

    x_d = nc.dram_tensor("x", [128, NP], dt.float16, kind="ExternalInput")
    w_d = nc.dram_tensor("w", [128, KS * 128], dt.float16, kind="ExternalInput")
    out_d = nc.dram_tensor("out", [D, H * W], dt.float16, kind="ExternalOutput")

    with tile.TileContext(nc) as tc:
        with tc.tile_pool(name="io", bufs=xp_bufs) as io_pool, \
             tc.tile_pool(name="wp", bufs=2) as w_pool, \
             tc.tile_pool(name="outp", bufs=out_bufs) as out_pool, \
             tc.tile_pool(name="stg", bufs=1) as stg_pool, \
             tc.tile_pool(name="ps", bufs=psum_bufs, space="PSUM") as ps_pool:

            for _rep in range(repeat):
                w_sb = w_pool.tile([128, KS * 128], dt.float16,
                                   name="w_sb", tag="w_sb")
                nc.sync.dma_start(w_sb[:, :], w_d.ap()[:, :])

                xp = io_pool.tile([128, NP], dt.float16, name="xp", tag="xp")
                bnd = [NP * g // in_chunks for g in range(in_chunks + 1)]
                if not skip_in:
                    for g in range(in_chunks):
                        a, b = bnd[g], bnd[g + 1]
                        nc.sync.dma_start(xp[:, a:b], x_d.ap()[:, a:b])
                else:
                    nc.sync.dma_start(xp[:, 0:NP], x_d.ap()[:, 0:NP]) if False else                     nc.sync.dma_start(xp[:, 0:64], x_d.ap()[:, 0:64])

                xv = xp.rearrange("p (r c) -> p r c", c=WP)
                outt = sbB = None
                if not skip_drain:
                    outt = out_pool.tile([D, H * W], dt.float16,
                                         name="outt", tag="outt")
                    sbB = stg_pool.tile([64, H * W + SHIFT], sdt,
                                        name="sbB", tag="sbB")

                ps_list = []
                for g in range(NGl):
                    if skip_mm:
                        break
                    psP = ps_pool.tile([128, Gl], mybir.dt.float32,
                                       name="psP", tag="psP")
                    ps_list.append(psP)
                    for s in range(spg if not skip_mm else 0):
                        h0 = TILE_ROWS * (spg * g + s)
                        for j in range(KS):
                            nc.tensor.matmul(
                                psP[:, TN * s:TN * (s + 1)],
                                lhsT=w_sb[:, 128 * j:128 * (j + 1)],
                                rhs=xv[:, h0:h0 + TILE_ROWS, j:j + W],
                                start=(j == 0), stop=(j == KS - 1),
                            )
                    # stage this group's beta partials contiguously
                    if skip_drain:
                        continue
                    nc.scalar.copy(sbB[:, Gl * g:Gl * (g + 1)], psP[64:128, :])
                    if g > 0:
                        nc.vector.tensor_add(
                            outt[:, Gl * (g - 1):Gl * g],
                            ps_list[g - 1][0:64, :],
                            sbB[:, Gl * (g - 1) + SHIFT:Gl * g + SHIFT])
                # cleanup: beta partials for the last 2 output rows
                psQ = None
                if not (skip_drain or skip_mm):
                    psQ = ps_pool.tile([128, Gl], mybir.dt.float32,
                                       name="psP", tag="psP")
                for j in range(KS if not (skip_drain or skip_mm) else 0):
                    nc.tensor.matmul(
                        psQ[0:64, 0:SHIFT],
                        lhsT=w_sb[0:64, 128 * j + 64:128 * j + 128],
                        rhs=xv[0:64, H:H + 2, j:j + W],
                        start=(j == 0), stop=(j == KS - 1),
                    )
                if not skip_drain:
                    nc.scalar.copy(sbB[:, H * W:H * W + SHIFT],
                                   psQ[0:64, 0:SHIFT])
                    nc.vector.tensor_add(
                        outt[:, Gl * (NGl - 1):Gl * NGl],
                        ps_list[NGl - 1][0:64, :],
                        sbB[:, Gl * (NGl - 1) + SHIFT:Gl * NGl + SHIFT])

                obnd = [H * W * g // out_chunks for g in range(out_chunks + 1)]
                osrc = xp[0:64, 0:H * W] if skip_drain else outt
                if not skip_out:
                    for g in range(out_chunks):
                        a, b = obnd[g], obnd[g + 1]
                        nc.scalar.dma_start(out_d.ap()[:, a:b], osrc[:, a:b])

    nc.compile()
    return nc


def _prep_inputs(x, weight, w_lin):
    w = np.asarray(weight).astype(np.float64)
    weff = w + (np.asarray(w_lin).astype(np.float64).T[:, None, :]
                - w.sum(axis=1, keepdims=True)) / 9.0
    weff = weff.astype(np.float32)                      # [C, 9, D]
    w_sb = np.zeros((128, KS * 128), np.float16)
    for j in range(KS):
        w_sb[0:C, 128 * j:128 * j + 64] = weff[:, 0 * KS + j, :]
        w_sb[C:128, 128 * j:128 * j + 64] = weff[:, 1 * KS + j, :]
        w_sb[0:C, 128 * j + 64:128 * j + 128] = weff[:, 2 * KS + j, :]

    xpad = np.pad(np.asarray(x), ((0, 0), (0, 0), (1, 1), (1, 1)), mode="edge")
    xpad = xpad.reshape(B, C, NP).astype(np.float16)
    xh = np.zeros((B, 128, NP), np.float16)
    xh[:, 0:C, :] = xpad
    xh[:, C:128, 0:NP - WP] = xpad[:, :, WP:]
    return xh, w_sb


def kernel(x, weight, w_lin):
    from concourse.bass_utils import run_bass_kernel_spmd

    if "nc" not in _CACHE:
        _CACHE["nc"] = _build()
    nc = _CACHE["nc"]

    xh, w_sb = _prep_inputs(x, weight, w_lin)
    in_maps = [{"x": xh[b], "w": w_sb} for b in range(B)]
    res = run_bass_kernel_spmd(nc, in_maps, core_ids=list(range(B)))
    out = np.stack([res.results[b]["out"].reshape(D, H, W) for b in range(B)])
    return out.astype(np.float32)


# revision 5
# speedup vs baseline: 1.3407x; 1.2544x over previous
"""Trainium2 Bass kernel for nn_CONV_A_64115271795341 — v4.

Same math as v3 (3 taps per matmul via [x; x>>row] contraction pairing plus
a beta column-group computing kernel-row-2 partials 2 output rows behind),
restructured for pipeline overlap:

  - psum groups of SPG=2 spans (2 banks) x 4 pool buffers = all 8 banks,
    giving 3 groups of slack between matmuls and the drain chain.
  - beta partials staged contiguously into sbB[64, H*W + 2W] (one ACT copy
    per group, partition-crossing 64:128 -> 0:64); the DVE add then reads
    a +2-row shifted window of sbB, no separate boundary ops.
  - cleanup matmuls (N=256) produce the last 2 rows' beta partials into a
    recycled psum buffer, staged into sbB's tail.
  - out[q] = psTop[q] + sbB[q+2 rows]; outt fp16, 2 big output DMAs.
"""

import numpy as np

C, H, W, D, B = 64, 128, 128, 64, 8
KS = 3
WP = W + 2            # 130
HP = H + 2
NP = WP * HP          # 16900
TILE_ROWS = 4
TN = TILE_ROWS * W    # 512
NSPANS = H // TILE_ROWS   # 32
SPG = 2                   # spans per psum group
NG = NSPANS // SPG        # 16 groups
G = SPG * TN              # 1024 cols per group
SHIFT = 2 * W             # 256: beta partials lag 2 output rows

_CACHE = {}


def _build(repeat=1, in_chunks=4, out_chunks=1, psum_bufs=4, xp_bufs=2,
           out_bufs=2, stg_dtype="float32", dup="host", dup_chunks=8,
           spg=SPG, skip_in=False, skip_out=False, skip_drain=False,
           skip_mm=False):
    NGl = NSPANS // spg
    Gl = spg * TN
    import concourse.bass as bass  # noqa: F401
    import concourse.mybir as mybir
    import concourse.tile as tile
    from concourse import bacc

    dt = mybir.dt
    sdt = getattr(dt, stg_dtype)
    nc = bacc.Bacc("TRN2", target_bir_lowering=False, debug=False, num_devices=8)

    x_d = nc.dram_tensor("x", [128, NP], dt.float16, kind="ExternalInput")
    w_d = nc.dram_tensor("w", [128, KS * 128], dt.float16, kind="ExternalInput")
    out_d = nc.dram_tensor("out", [D, H * W], dt.float16, kind="ExternalOutput")

    with tile.TileContext(nc) as tc:
        with tc.tile_pool(name="io", bufs=xp_bufs) as io_pool, \
             tc.tile_pool(name="wp", bufs=2) as w_pool, \
             tc.tile_pool(name="outp", bufs=out_bufs) as out_pool, \
             tc.tile_pool(name="stg", bufs=1) as stg_pool, \
             tc.tile_pool(name="ps", bufs=psum_bufs, space="PSUM") as ps_pool:

            for _rep in range(repeat):
                w_sb = w_pool.tile([128, KS * 128], dt.float16,
                                   name="w_sb", tag="w_sb")
                nc.sync.dma_start(w_sb[:, :], w_d.ap()[:, :])

                xp = io_pool.tile([128, NP], dt.float16, name="xp", tag="xp")
                bnd = [NP * g // in_chunks for g in range(in_chunks + 1)]
                if not skip_in:
                    for g in range(in_chunks):
                        a, b = bnd[g], bnd[g + 1]
                        nc.sync.dma_start(xp[:, a:b], x_d.ap()[:, a:b])
                else:
                    nc.sync.dma_start(xp[:, 0:NP], x_d.ap()[:, 0:NP]) if False else                     nc.sync.dma_start(xp[:, 0:64], x_d.ap()[:, 0:64])

                xv = xp.rearrange("p (r c) -> p r c", c=WP)
                outt = sbB = None
                if not skip_drain:
                    outt = out_pool.tile([D, H * W], dt.float16,
                                         name="outt", tag="outt")
                    sbB = stg_pool.tile([64, H * W + W], sdt,
                                        name="sbB", tag="sbB")

                # head piece: beta (tap-row-0) partials for output row 0,
                # computed up front from x row 0 (available with chunk 0)
                if not (skip_drain or skip_mm):
                    psQ = ps_pool.tile([128, Gl], mybir.dt.float32,
                                       name="psP", tag="psP")
                    for j in range(KS):
                        nc.tensor.matmul(
                            psQ[0:64, 0:W],
                            lhsT=w_sb[0:64, 128 * j + 64:128 * j + 128],
                            rhs=xv[0:64, 0:1, j:j + W],
                            start=(j == 0), stop=(j == KS - 1),
                        )
                    nc.scalar.copy(sbB[:, 0:W], psQ[0:64, 0:W])

                ps_list = []
                for g in range(NGl):
                    if skip_mm:
                        break
                    psP = ps_pool.tile([128, Gl], mybir.dt.float32,
                                       name="psP", tag="psP")
                    ps_list.append(psP)
                    for s in range(spg if not skip_mm else 0):
                        h0 = TILE_ROWS * (spg * g + s)
                        for j in range(KS):
                            nc.tensor.matmul(
                                psP[:, TN * s:TN * (s + 1)],
                                lhsT=w_sb[:, 128 * j:128 * (j + 1)],
                                rhs=xv[:, h0 + 1:h0 + 1 + TILE_ROWS, j:j + W],
                                start=(j == 0), stop=(j == KS - 1),
                            )
                    # stage this group's beta partials contiguously
                    if skip_drain:
                        continue
                    nc.scalar.copy(sbB[:, W + Gl * g:W + Gl * (g + 1)],
                                   psP[64:128, :])
                    nc.vector.tensor_add(
                        outt[:, Gl * g:Gl * (g + 1)],
                        psP[0:64, :],
                        sbB[:, Gl * g:Gl * (g + 1)])

                obnd = [H * W * g // out_chunks for g in range(out_chunks + 1)]
                osrc = xp[0:64, 0:H * W] if skip_drain else outt
                if not skip_out:
                    for g in range(out_chunks):
                        a, b = obnd[g], obnd[g + 1]
                        nc.scalar.dma_start(out_d.ap()[:, a:b], osrc[:, a:b])

    nc.compile()
    return nc


def _prep_inputs(x, weight, w_lin):
    w = np.asarray(weight).astype(np.float64)
    weff = w + (np.asarray(w_lin).astype(np.float64).T[:, None, :]
                - w.sum(axis=1, keepdims=True)) / 9.0
    weff = weff.astype(np.float32)                      # [C, 9, D]
    w_sb = np.zeros((128, KS * 128), np.float16)
    for j in range(KS):
        w_sb[0:C, 128 * j:128 * j + 64] = weff[:, 1 * KS + j, :]
        w_sb[C:128, 128 * j:128 * j + 64] = weff[:, 2 * KS + j, :]
        w_sb[0:C, 128 * j + 64:128 * j + 128] = weff[:, 0 * KS + j, :]

    xpad = np.pad(np.asarray(x), ((0, 0), (0, 0), (1, 1), (1, 1)), mode="edge")
    xpad = xpad.reshape(B, C, NP).astype(np.float16)
    xh = np.zeros((B, 128, NP), np.float16)
    xh[:, 0:C, :] = xpad
    xh[:, C:128, 0:NP - WP] = xpad[:, :, WP:]
    return xh, w_sb


def kernel(x, weight, w_lin):
    from concourse.bass_utils import run_bass_kernel_spmd

    if "nc" not in _CACHE:
        _CACHE["nc"] = _build()
    nc = _CACHE["nc"]

    xh, w_sb = _prep_inputs(x, weight, w_lin)
    in_maps = [{"x": xh[b], "w": w_sb} for b in range(B)]
    res = run_bass_kernel_spmd(nc, in_maps, core_ids=list(range(B)))
    out = np.stack([res.results[b]["out"].reshape(D, H, W) for b in range(B)])
    return out.astype(np.float32)
